# revision 31
# baseline (speedup 1.0000x reference)
"""Causal single-head attention (B=4, S=4096, D=1024) on 8 TRN2 NeuronCores.

Sharding: core = (batch b, half h).  Each core computes attention output for
2048 queries of one batch: query chunks {0,3,4,7} (h=0) or {1,2,5,6} (h=1) of
8x512, which balances causal work.  Each core projects K^T/V for its full
batch; everything stays SBUF-resident (no DRAM scratch).

Mixed precision (validated in numpy + CoreSim, rel ~2.5e-3 vs 2e-2 gate):
  - fp16 island: scores for (q<512, k<512); ctx P/V fp16 only for slot 0
    (queries<1024) x keys<512.  Protects early (few-key) rows where softmax
    averaging is weak; all other queries are averaging-protected.
  - everything else: fp8 e4m3 with DoubleRow matmuls (2x PE throughput).
Scale folding (dodges e4m3 subnormals/overflow):
  Wq8,Wk8 scaled x32 -> s8 = 1024*s -> exp scale 1/32768
  Wv8 scaled x16; P8 stored as p/16 via exp bias -ln(16); den repaired with
  ones8=16; fp16 paths are true-scale.
Denominator: ones-column matmuls with full-partition lhsT produce den
replicated over all 128 partitions (no separate broadcast matmul needed).
Layouts (all SBUF):
  K^T  : KT8 4x[P,8,1024] f8, KT16 [P,8,512] f16
  Q^T  : Q8 4x[P,8,512] f8 per slot, Q16 [P,8,512] f16 (slot 0)
  V    : V8 [P,32,1024] f8 (all tiles), V16 [P,4,1024] f16 (tiles 0-3)
  P    : P16 [P,4,512] f16 (slot 0 tiles 0-3), P8 pairs [P,2,512] f8
  scores^T = [k, q]: psum = sum_d KT[d,k128].T @ QT[d,q512] (no transposes)
"""

import math
import sys

for _p in ("/opt/trn_rl_repo",):
    if _p not in sys.path:
        sys.path.insert(0, _p)

import numpy as np
import ml_dtypes

B, S, D = 4, 4096, 1024
P = 128
CH = 512                       # query chunk
NSLOT = 4                      # chunks per core
NQ = NSLOT * CH                # queries per core
NK = [8, 16, 24, 32]           # k-tiles per slot (uniform across cores)
CHUNKS_H = [[0, 3, 4, 7], [1, 2, 5, 6]]
SC16 = 1.0 / 32.0              # 1/sqrt(D)
SC8 = 1.0 / 32768.0            # 1/sqrt(D) / (32*32)
BIAS8 = -math.log(16.0)        # P8 stored as p/16
F8 = ml_dtypes.float8_e4m3

_PROGRAM = None


def _build_program():
    import concourse.bass as bass
    import concourse.tile as tile
    import concourse.mybir as mybir
    from concourse import bacc
    from concourse.bass import ds, ts

    f32 = mybir.dt.float32
    f16 = mybir.dt.float16
    f8 = mybir.dt.float8e4
    DR = mybir.MatmulPerfMode.DoubleRow

    nc = bacc.Bacc(trn_type="TRN2", target_bir_lowering=False, debug=False,
                   num_devices=8)

    xc16_d = nc.declare_dram_parameter("xc16", [P, 8, CH], f16, isOutput=False)
    x8_d = nc.declare_dram_parameter("x8", [7, P, 8, CH], f8, isOutput=False)
    xc8_d = nc.declare_dram_parameter("xc8", [P, 8, CH], f8, isOutput=False)
    xq16_d = nc.declare_dram_parameter("xq16", [P, 8, CH], f16, isOutput=False)
    xq8_d = nc.declare_dram_parameter("xq8", [3, P, 8, CH], f8, isOutput=False)
    wk16_d = nc.declare_dram_parameter("wk16", [P, 8, D], f16, isOutput=False)
    wv16_d = nc.declare_dram_parameter("wv16", [P, 8, D], f16, isOutput=False)
    wq16_d = nc.declare_dram_parameter("wq16", [P, 8, D], f16, isOutput=False)
    wk8_d = nc.declare_dram_parameter("wk8", [P, 8, D], f8, isOutput=False)
    wv8_d = nc.declare_dram_parameter("wv8", [P, 8, D], f8, isOutput=False)
    wq8_d = nc.declare_dram_parameter("wq8", [P, 8, D], f8, isOutput=False)
    amat_d = nc.declare_dram_parameter("amat", [P, 32], f16, isOutput=False)
    bias8_d = nc.declare_dram_parameter("bias8", [P, 1], f32, isOutput=False)
    dmat_d = nc.declare_dram_parameter("dmat", [P, CH], f16, isOutput=False)
    ones16_d = nc.declare_dram_parameter("ones16", [P, P], f16, isOutput=False)
    ones8_d = nc.declare_dram_parameter("ones8", [P, 2, P], f8, isOutput=False)
    outT = nc.declare_dram_parameter("outT", [D, NQ], f32, isOutput=True)

    Exp = mybir.ActivationFunctionType.Exp
    Copy = mybir.ActivationFunctionType.Copy
    is_le = mybir.AluOpType.is_le
    mult = mybir.AluOpType.mult

    with tile.TileContext(nc, pool_alloc_mode="queue") as tc:
        with (
            tc.tile_pool(name="kt", bufs=1) as kt_pool,
            tc.tile_pool(name="qt", bufs=1) as qt_pool,
            tc.tile_pool(name="vt", bufs=1) as vt_pool,
            tc.tile_pool(name="const", bufs=1) as const_pool,
        ):
            KT8 = [
                kt_pool.tile([P, 8, 1024], f8, tag=f"kt{i}", name=f"KT8_{i}")
                for i in range(4)
            ]
            KT16 = kt_pool.tile([P, 8, CH], f16, tag="kt16", name="KT16")
            Q8 = [
                qt_pool.tile([P, 8, CH], f8, tag=f"qt{i}", name=f"Q8_{i}")
                for i in range(NSLOT)
            ]
            Q16 = qt_pool.tile([P, 8, CH], f16, tag="qt16", name="Q16")
            V16 = vt_pool.tile([P, 2, D], f16, tag="v16", name="V16")
            V8 = vt_pool.tile([P, 32, D], f8, tag="v8", name="V8")
            dmat = const_pool.tile([P, CH], f16, tag="dmat")
            amat = const_pool.tile([P, 32], f16, tag="amat")
            ones16 = const_pool.tile([P, P], f16, tag="ones16")
            ones8 = const_pool.tile([P, 2, P], f8, tag="ones8")
            bias8 = const_pool.tile([P, 1], f32, tag="bias8")
            nc.gpsimd.dma_start(out=dmat[:], in_=dmat_d[:])
            nc.gpsimd.dma_start(out=amat[:], in_=amat_d[:])
            nc.gpsimd.dma_start(out=bias8[:], in_=bias8_d[:])
            nc.gpsimd.dma_start(out=ones16[:], in_=ones16_d[:])
            nc.gpsimd.dma_start(out=ones8[:], in_=ones8_d[:])

            # ---------- Phase A: projections ------------------------------
            with (
                tc.tile_pool(name="w16", bufs=3) as w16_pool,
                tc.tile_pool(name="w8", bufs=1) as w8_pool,
                tc.tile_pool(name="x16", bufs=1) as x16_pool,
                tc.tile_pool(name="x8", bufs=2) as x8_pool,
                tc.tile_pool(name="xq8", bufs=2) as xq8_pool,
                tc.tile_pool(name="ps0", bufs=8, space="PSUM") as ps_pool,
            ):
                wk16 = w16_pool.tile([P, 8, D], f16, tag="w16", name="wk16")
                wk8 = w8_pool.tile([P, 8, D], f8, tag="wk8")
                wv8 = w8_pool.tile([P, 8, D], f8, tag="wv8")
                wq8 = w8_pool.tile([P, 8, D], f8, tag="wq8")
                xc16 = x16_pool.tile([P, 8, CH], f16, tag="xc16")
                xq16 = x16_pool.tile([P, 8, CH], f16, tag="xq16")

                # slab-split loads so the first matmuls start early.
                # Order: Q16 runs first (wq16+xq16 land first on their
                # queues), then K island (wk16), then V island (wv16 reuses
                # wq16's ring slot once Q16 is done).
                xc8 = x8_pool.tile([P, 8, CH], f8, tag="xc", name="xc8")
                wq16 = w16_pool.tile([P, 8, D], f16, tag="w16", name="wq16")
                wv16 = w16_pool.tile([P, 8, D], f16, tag="w16", name="wv16")
                # fp8 pass runs first: its small loads go ahead of the bulky
                # fp16 weights, which stream during the ~40us of fp8 compute
                nc.sync.dma_start(out=wk8[:], in_=wk8_d[:])
                nc.gpsimd.dma_start(out=wv8[:], in_=wv8_d[:])
                nc.scalar.dma_start(out=xc8[:], in_=xc8_d[:])
                for d in range(8):
                    nc.sync.dma_start(out=wq16[:, d, :], in_=wq16_d[:, d, :])
                    nc.scalar.dma_start(out=xq16[:, d, :], in_=xq16_d[:, d, :])
                for d in range(8):
                    nc.sync.dma_start(out=wk16[:, d, :], in_=wk16_d[:, d, :])
                    nc.gpsimd.dma_start(out=xc16[:, d, :], in_=xc16_d[:, d, :])
                for d in range(8):
                    nc.sync.dma_start(out=wv16[:, d, :], in_=wv16_d[:, d, :])
                nc.scalar.dma_start(out=wq8[:], in_=wq8_d[:])

                # fp8 chunks 1-3 first (cheap loads; fp16 weights stream)
                for c in range(1, 4):
                    xc = x8_pool.tile([P, 8, CH], f8, tag="xc", name=f"xc{c}")
                    nc.sync.dma_start(out=xc[:], in_=x8_d[c - 1])
                    for o in range(8):
                        ps = ps_pool.tile([P, CH], f32, tag="ps", name="psk8")
                        for d2 in range(4):
                            nc.tensor.matmul(
                                ps[:],
                                lhsT=wk8[:, ds(2 * d2, 2), ts(o, P)],
                                rhs=xc[:, ds(2 * d2, 2), :],
                                start=(d2 == 0),
                                stop=(d2 == 3),
                                perf_mode=DR,
                            )
                        nc.vector.tensor_copy(
                            KT8[c // 2][:, o, ds((c % 2) * CH, CH)], ps[:]
                        )
                    for kt in range(4):
                        for oh in range(2):
                            ps = ps_pool.tile([P, CH], f32, tag="ps", name="psv8")
                            for d2 in range(4):
                                nc.tensor.matmul(
                                    ps[:],
                                    lhsT=xc[:, ds(2 * d2, 2), ts(kt, P)],
                                    rhs=wv8[:, ds(2 * d2, 2), ts(oh, CH)],
                                    start=(d2 == 0),
                                    stop=(d2 == 3),
                                    perf_mode=DR,
                                )
                            nc.scalar.copy(
                                V8[:, 4 * c + kt, ts(oh, CH)], ps[:]
                            )

                # Q16 projection first (fp16 slot 0, dual store).
                # d-outer over 8 psum banks: each arriving wq16 slab feeds 8
                # matmuls, so the PE starts after slab 0 instead of pacing
                # behind the whole weight load.
                qps = [
                    ps_pool.tile([P, CH], f32, tag="ps", name=f"psq16_{o}")
                    for o in range(8)
                ]
                for d in range(8):
                    for o in range(8):
                        nc.tensor.matmul(
                            qps[o][:],
                            lhsT=wq16[:, d, ts(o, P)],
                            rhs=xq16[:, d, :],
                            start=(d == 0),
                            stop=(d == 7),
                        )
                for o in range(8):
                    nc.vector.tensor_copy(Q16[:, o, :], qps[o][:])
                    nc.scalar.activation(
                        Q8[0][:, o, :], qps[o][:], Copy, scale=32.0
                    )

                # fp16 chunk-0 K (dual store: f16 true + f8 x32)
                for o in range(8):
                    ps = ps_pool.tile([P, CH], f32, tag="ps", name="psk16")
                    for d in range(8):
                        nc.tensor.matmul(
                            ps[:],
                            lhsT=wk16[:, d, ts(o, P)],
                            rhs=xc16[:, d, :],
                            start=(d == 0),
                            stop=(d == 7),
                        )
                    nc.vector.tensor_copy(KT16[:, o, :], ps[:])
                    nc.scalar.activation(
                        KT8[0][:, o, ds(0, CH)], ps[:], Copy, scale=32.0
                    )
                # chunk-0 V: kt 0-1 fp16 (dual store f16 + f8 x16),
                # kt 2-3 fp8 DoubleRow (fp8 x, fp8 w)
                for kt in range(2):
                    for oh in range(2):
                        ps = ps_pool.tile([P, CH], f32, tag="ps", name="psv16")
                        for d in range(8):
                            nc.tensor.matmul(
                                ps[:],
                                lhsT=xc16[:, d, ts(kt, P)],
                                rhs=wv16[:, d, ts(oh, CH)],
                                start=(d == 0),
                                stop=(d == 7),
                            )
                        nc.vector.tensor_copy(V16[:, kt, ts(oh, CH)], ps[:])
                        nc.scalar.activation(
                            V8[:, kt, ts(oh, CH)], ps[:], Copy, scale=16.0
                        )
                for kt in range(2, 4):
                    for oh in range(2):
                        ps = ps_pool.tile([P, CH], f32, tag="ps", name="psv8c0")
                        for d2 in range(4):
                            nc.tensor.matmul(
                                ps[:],
                                lhsT=xc8[:, ds(2 * d2, 2), ts(kt, P)],
                                rhs=wv8[:, ds(2 * d2, 2), ts(oh, CH)],
                                start=(d2 == 0),
                                stop=(d2 == 3),
                                perf_mode=DR,
                            )
                        nc.scalar.copy(V8[:, kt, ts(oh, CH)], ps[:])

                # fp8 chunks 4-7
                for c in range(4, 8):
                    xc = x8_pool.tile([P, 8, CH], f8, tag="xc", name=f"xc{c}")
                    nc.sync.dma_start(out=xc[:], in_=x8_d[c - 1])
                    for o in range(8):
                        ps = ps_pool.tile([P, CH], f32, tag="ps", name="psk8")
                        for d2 in range(4):
                            nc.tensor.matmul(
                                ps[:],
                                lhsT=wk8[:, ds(2 * d2, 2), ts(o, P)],
                                rhs=xc[:, ds(2 * d2, 2), :],
                                start=(d2 == 0),
                                stop=(d2 == 3),
                                perf_mode=DR,
                            )
                        nc.vector.tensor_copy(
                            KT8[c // 2][:, o, ds((c % 2) * CH, CH)], ps[:]
                        )
                    for kt in range(4):
                        for oh in range(2):
                            ps = ps_pool.tile([P, CH], f32, tag="ps", name="psv8")
                            for d2 in range(4):
                                nc.tensor.matmul(
                                    ps[:],
                                    lhsT=xc[:, ds(2 * d2, 2), ts(kt, P)],
                                    rhs=wv8[:, ds(2 * d2, 2), ts(oh, CH)],
                                    start=(d2 == 0),
                                    stop=(d2 == 3),
                                    perf_mode=DR,
                                )
                            nc.scalar.copy(
                                V8[:, 4 * c + kt, ts(oh, CH)], ps[:]
                            )

                # Q projections: fp8 slots 1-3
                for sl in range(1, 4):
                    xq = xq8_pool.tile([P, 8, CH], f8, tag="xq", name=f"xq{sl}")
                    nc.scalar.dma_start(out=xq[:], in_=xq8_d[sl - 1])
                    for o in range(8):
                        ps = ps_pool.tile([P, CH], f32, tag="ps", name="psq8")
                        for d2 in range(4):
                            nc.tensor.matmul(
                                ps[:],
                                lhsT=wq8[:, ds(2 * d2, 2), ts(o, P)],
                                rhs=xq[:, ds(2 * d2, 2), :],
                                start=(d2 == 0),
                                stop=(d2 == 3),
                                perf_mode=DR,
                            )
                        nc.vector.tensor_copy(Q8[sl][:, o, :], ps[:])

            # ---------------- Phase B: attention --------------------------
            with (
                tc.tile_pool(name="p16", bufs=2) as p16_pool,
                tc.tile_pool(name="p8", bufs=30) as p8_pool,
                tc.tile_pool(name="et", bufs=6) as e_pool,
                tc.tile_pool(name="fo", bufs=6) as f_pool,
                tc.tile_pool(name="pss", bufs=4, space="PSUM") as s_ps_pool,
                tc.tile_pool(name="psc", bufs=3, space="PSUM") as c_ps_pool,
                tc.tile_pool(name="psd", bufs=1, space="PSUM") as d_ps_pool,
            ):
                for s in range(NSLOT):
                    nk = NK[s]
                    n16 = 2 if s == 0 else 0   # fp16 P/V tiles (slot 0 only)
                    np8 = (nk - n16) // 2
                    P16 = (
                        p16_pool.tile([P, 2, CH], f16, tag="p16", name="P16")
                        if n16
                        else None
                    )
                    P8 = [
                        p8_pool.tile([P, 2, CH], f8, tag="p8", name=f"P8_{s}_{t}")
                        for t in range(np8)
                    ]
                    jorder = (
                        list(range(nk))
                        if s == 0
                        else list(range(nk - 8, nk)) + list(range(nk - 8))
                    )
                    for j in jorder:
                        mm16 = (s == 0 and j < 4)
                        # Diagonal tiles (last 4 of each slot; tiles 4-7 in
                        # slot 0): queries below the tile start are masked,
                        # so skip those score columns and zero the P region.
                        di = -1
                        if s == 0 and j >= 4:
                            di = j - 4
                        elif s > 0 and j >= nk - 4:
                            di = j - (nk - 4)
                        qlo = P * di if di > 0 else 0
                        CHq = CH - qlo
                        sps = s_ps_pool.tile([P, CH], f32, name="sps")
                        if mm16:
                            for o in range(8):
                                nc.tensor.matmul(
                                    sps[:],
                                    lhsT=KT16[:, o, ds(j * P, P)],
                                    rhs=Q16[:, o, :],
                                    start=(o == 0),
                                    stop=(o == 7),
                                )
                        else:
                            for d2 in range(4):
                                nc.tensor.matmul(
                                    sps[:, ds(qlo, CHq)],
                                    lhsT=KT8[j // 8][
                                        :, ds(2 * d2, 2), ds((j % 8) * P, P)
                                    ],
                                    rhs=Q8[s][:, ds(2 * d2, 2), ds(qlo, CHq)],
                                    start=(d2 == 0),
                                    stop=(d2 == 3),
                                    perf_mode=DR,
                                )
                        scale = SC16 if mm16 else SC8
                        if j < n16:
                            dst = P16[:, j, :]
                            bias = 0.0
                        else:
                            dst = P8[(j - n16) // 2][:, (j - n16) % 2, :]
                            bias = bias8[:]
                        stt = (s == 0) or (j >= nk - 8)
                        if qlo:
                            nc.gpsimd.memset(dst[:, ds(0, qlo)], 0.0)
                            dst = dst[:, ds(qlo, CHq)]
                        if stt:
                            et = e_pool.tile([P, CH], f16, tag="et", name="et")
                            nc.scalar.activation(
                                et[:, ds(qlo, CHq)],
                                sps[:, ds(qlo, CHq)],
                                Exp,
                                scale=scale,
                                bias=bias,
                            )
                            col = j if s == 0 else 8 * s + (j - (nk - 8))
                            nc.vector.scalar_tensor_tensor(
                                out=dst,
                                in0=dmat[:, ds(qlo, CHq)],
                                scalar=amat[:, ds(col, 1)],
                                in1=et[:, ds(qlo, CHq)],
                                op0=is_le,
                                op1=mult,
                            )
                        else:
                            nc.scalar.activation(
                                dst, sps[:], Exp, scale=scale, bias=bias
                            )
                    # denominator, replicated across partitions by the
                    # full-width ones lhsT (no broadcast matmul needed)
                    dps = d_ps_pool.tile([P, CH], f32, name="dps")
                    for j in range(n16):
                        nc.tensor.matmul(
                            dps[:],
                            lhsT=ones16[:],
                            rhs=P16[:, j, :],
                            start=(j == 0),
                            stop=False,
                        )
                    for t in range(np8):
                        # last pair (tiles nk-2/nk-1): P is zero for q<256
                        ql = CH // 2 if t == np8 - 1 else 0
                        nc.tensor.matmul(
                            dps[:, ds(ql, CH - ql)],
                            lhsT=ones8[:],
                            rhs=P8[t][:, :, ds(ql, CH - ql)],
                            start=(n16 == 0 and t == 0),
                            stop=(t == np8 - 1),
                            perf_mode=DR,
                        )
                    rec = f_pool.tile([P, CH], f32, tag="rec", name="rec")
                    nc.vector.reciprocal(rec[:], dps[:])
                    # context
                    for o in range(8):
                        cps = c_ps_pool.tile([P, CH], f32, name="cps")
                        for j in range(n16):
                            nc.tensor.matmul(
                                cps[:],
                                lhsT=V16[:, j, ts(o, P)],
                                rhs=P16[:, j, :],
                                start=(j == 0),
                                stop=False,
                            )
                        for t in range(np8):
                            ql = CH // 2 if t == np8 - 1 else 0
                            nc.tensor.matmul(
                                cps[:, ds(ql, CH - ql)],
                                lhsT=V8[:, ds(n16 + 2 * t, 2), ts(o, P)],
                                rhs=P8[t][:, :, ds(ql, CH - ql)],
                                start=(n16 == 0 and t == 0),
                                stop=(t == np8 - 1),
                                perf_mode=DR,
                            )
                        ft = f_pool.tile([P, CH], f32, tag="ft", name="ft")
                        nc.vector.tensor_mul(ft[:], cps[:], rec[:])
                        qeng = (nc.sync, nc.scalar, nc.gpsimd)[o % 3]
                        qeng.dma_start(
                            out=outT[ds(o * P, P), ts(s, CH)], in_=ft[:]
                        )

    nc.compile()
    return nc


def _get_program():
    global _PROGRAM
    if _PROGRAM is None:
        _PROGRAM = _build_program()
    return _PROGRAM


def _tile_w(w, scale, dtype):
    # [o, i] -> [p, d_slab, o] with d = d_slab*128 + p
    wt = (np.asarray(w, dtype=np.float32).T * scale).astype(dtype)
    return np.ascontiguousarray(wt.reshape(8, P, D).transpose(1, 0, 2))


def _tile_x(xt, dtype):
    # [d, s_cols] -> [p, d_slab, s] (or [n, p, d_slab, s] for multi-chunk)
    ncols = xt.shape[1]
    nch = ncols // CH
    t = np.ascontiguousarray(
        xt.reshape(8, P, nch, CH).transpose(2, 1, 0, 3)
    ).astype(dtype)
    if nch == 1:
        return np.ascontiguousarray(t[0])
    return t


def _make_in_maps(x, W_query, W_key, W_value):
    xT = np.asarray(x, dtype=np.float32).transpose(0, 2, 1)  # [B, D, S]

    wk16 = _tile_w(W_key, 1.0, np.float16)
    wv16 = _tile_w(W_value, 1.0, np.float16)
    wq16 = _tile_w(W_query, 1.0, np.float16)
    wk8 = _tile_w(W_key, 32.0, F8)
    wv8 = _tile_w(W_value, 16.0, F8)
    wq8 = _tile_w(W_query, 32.0, F8)

    dmat = (
        np.arange(P, dtype=np.float32)[:, None]
        - np.arange(CH, dtype=np.float32)[None, :]
    ).astype(np.float16)
    amat_h = []
    for h in range(2):
        a = np.full((P, 32), -32768.0, np.float32)
        for sl in range(NSLOT):
            cid = CHUNKS_H[h][sl]
            nk = NK[sl]
            if sl == 0:
                for j in range(8):
                    a[:, j] = CH * cid - P * j
            else:
                for j in range(nk - 8, nk):
                    a[:, 8 * sl + (j - (nk - 8))] = CH * cid - P * j
        amat_h.append(np.ascontiguousarray(a.astype(np.float16)))
    ones16 = np.ones((P, P), np.float16)
    ones8 = np.full((P, 2, P), 16.0, F8)
    bias8 = np.full((P, 1), BIAS8, np.float32)

    in_maps = []
    for core in range(8):
        b, h = core // 2, core % 2
        xb = xT[b]
        q0 = CHUNKS_H[h][0]
        xq_cols = np.concatenate(
            [np.arange(c * CH, (c + 1) * CH) for c in CHUNKS_H[h][1:]]
        )
        in_maps.append(
            {
                "xc16": _tile_x(xb[:, :CH], np.float16),
                "xc8": _tile_x(xb[:, :CH], F8),
                "x8": _tile_x(xb[:, CH:], F8),
                "xq16": _tile_x(xb[:, q0 * CH : (q0 + 1) * CH], np.float16),
                "xq8": _tile_x(np.ascontiguousarray(xb[:, xq_cols]), F8),
                "wk16": wk16,
                "wv16": wv16,
                "wq16": wq16,
                "wk8": wk8,
                "wv8": wv8,
                "wq8": wq8,
                "amat": amat_h[h],
                "dmat": dmat,
                "ones16": ones16,
                "ones8": ones8,
                "bias8": bias8,
            }
        )
    return in_maps


def _assemble(results):
    out = np.empty((B, S, D), np.float32)
    for core in range(8):
        b, h = core // 2, core % 2
        oT = np.asarray(results[core]["outT"])  # [D, NQ]
        for slot, c in enumerate(CHUNKS_H[h]):
            out[b, c * CH : (c + 1) * CH, :] = oT[:, slot * CH : (slot + 1) * CH].T
    return out


def run(inputs, trace=False, trace_cores=None):
    """Run the kernel; returns (output, BassKernelResults)."""
    from concourse.bass_utils import run_bass_kernel_spmd

    nc = _get_program()
    in_maps = _make_in_maps(
        inputs["x"], inputs["W_query"], inputs["W_key"], inputs["W_value"]
    )
    kw = {}
    if trace:
        kw = dict(trace=True, trace_cores=trace_cores, stitch_traces=False)
    res = run_bass_kernel_spmd(nc, in_maps, list(range(8)), **kw)
    return _assemble(res.results), res


def kernel(x, W_query, W_key, W_value):
    out, _ = run({"x": x, "W_query": W_query, "W_key": W_key, "W_value": W_value})
    return out


# revision 33
# speedup vs baseline: 1.1206x; 1.1206x over previous
"""Causal single-head attention (B=4, S=4096, D=1024) on 8 TRN2 NeuronCores.

Sharding: core = (batch b, half h).  Each core computes attention output for
2048 queries of one batch: query chunks {0,3,4,7} (h=0) or {1,2,5,6} (h=1) of
8x512, which balances causal work.  Each core projects K^T/V for its full
batch; everything stays SBUF-resident (no DRAM scratch).

Mixed precision (validated in numpy + CoreSim, rel ~2.5e-3 vs 2e-2 gate):
  - fp16 island: scores for (q<512, k<512); ctx P/V fp16 only for slot 0
    (queries<1024) x keys<512.  Protects early (few-key) rows where softmax
    averaging is weak; all other queries are averaging-protected.
  - everything else: fp8 e4m3 with DoubleRow matmuls (2x PE throughput).
Scale folding (dodges e4m3 subnormals/overflow):
  Wq8,Wk8 scaled x32 -> s8 = 1024*s -> exp scale 1/32768
  Wv8 scaled x16; P8 stored as p/16 via exp bias -ln(16); den repaired with
  ones8=16; fp16 paths are true-scale.
Denominator: ones-column matmuls with full-partition lhsT produce den
replicated over all 128 partitions (no separate broadcast matmul needed).
Layouts (all SBUF):
  K^T  : KT8 4x[P,8,1024] f8, KT16 [P,8,512] f16
  Q^T  : Q8 4x[P,8,512] f8 per slot, Q16 [P,8,512] f16 (slot 0)
  V    : V8 [P,32,1024] f8 (all tiles), V16 [P,4,1024] f16 (tiles 0-3)
  P    : P16 [P,4,512] f16 (slot 0 tiles 0-3), P8 pairs [P,2,512] f8
  scores^T = [k, q]: psum = sum_d KT[d,k128].T @ QT[d,q512] (no transposes)
"""

import math
import sys

for _p in ("/opt/trn_rl_repo",):
    if _p not in sys.path:
        sys.path.insert(0, _p)

import numpy as np
import ml_dtypes

B, S, D = 4, 4096, 1024
P = 128
CH = 512                       # query chunk
NSLOT = 4                      # chunks per core
NQ = NSLOT * CH                # queries per core
NK = [8, 16, 24, 32]           # k-tiles per slot (uniform across cores)
CHUNKS_H = [[0, 3, 4, 7], [1, 2, 5, 6]]
SC16 = 1.0 / 32.0              # 1/sqrt(D)
SC8 = 1.0 / 32768.0            # 1/sqrt(D) / (32*32)
BIAS8 = -math.log(16.0)        # P8 stored as p/16
F8 = ml_dtypes.float8_e4m3

_PROGRAM = None


def _build_program():
    import concourse.bass as bass
    import concourse.tile as tile
    import concourse.mybir as mybir
    from concourse import bacc
    from concourse.bass import ds, ts

    f32 = mybir.dt.float32
    f16 = mybir.dt.float16
    f8 = mybir.dt.float8e4
    DR = mybir.MatmulPerfMode.DoubleRow

    nc = bacc.Bacc(trn_type="TRN2", target_bir_lowering=False, debug=False,
                   num_devices=8)

    xc16_d = nc.declare_dram_parameter("xc16", [P, 8, CH], f16, isOutput=False)
    x8_d = nc.declare_dram_parameter("x8", [7, P, 8, CH], f8, isOutput=False)
    xc8_d = nc.declare_dram_parameter("xc8", [P, 8, CH], f8, isOutput=False)
    xq16_d = nc.declare_dram_parameter("xq16", [P, 8, CH], f16, isOutput=False)
    xq8_d = nc.declare_dram_parameter("xq8", [3, P, 8, CH], f8, isOutput=False)
    wk16_d = nc.declare_dram_parameter("wk16", [P, 8, D], f16, isOutput=False)
    wv16_d = nc.declare_dram_parameter("wv16", [P, 8, D], f16, isOutput=False)
    wq16_d = nc.declare_dram_parameter("wq16", [P, 8, D], f16, isOutput=False)
    wk8_d = nc.declare_dram_parameter("wk8", [P, 8, D], f8, isOutput=False)
    wv8_d = nc.declare_dram_parameter("wv8", [P, 8, D], f8, isOutput=False)
    wq8_d = nc.declare_dram_parameter("wq8", [P, 8, D], f8, isOutput=False)
    amat_d = nc.declare_dram_parameter("amat", [P, 32], f16, isOutput=False)
    bias8_d = nc.declare_dram_parameter("bias8", [P, 1], f32, isOutput=False)
    dmat_d = nc.declare_dram_parameter("dmat", [P, CH], f16, isOutput=False)
    ones16_d = nc.declare_dram_parameter("ones16", [P, P], f16, isOutput=False)
    ones8_d = nc.declare_dram_parameter("ones8", [P, 2, P], f8, isOutput=False)
    outT = nc.declare_dram_parameter("outT", [D, NQ], f32, isOutput=True)

    Exp = mybir.ActivationFunctionType.Exp
    Copy = mybir.ActivationFunctionType.Copy
    is_le = mybir.AluOpType.is_le
    mult = mybir.AluOpType.mult

    with tile.TileContext(nc, pool_alloc_mode="queue") as tc:
        with (
            tc.tile_pool(name="kt", bufs=1) as kt_pool,
            tc.tile_pool(name="qt", bufs=1) as qt_pool,
            tc.tile_pool(name="vt", bufs=1) as vt_pool,
            tc.tile_pool(name="const", bufs=1) as const_pool,
        ):
            KT8 = [
                kt_pool.tile([P, 8, 1024], f8, tag=f"kt{i}", name=f"KT8_{i}")
                for i in range(4)
            ]
            KT16 = kt_pool.tile([P, 8, CH], f16, tag="kt16", name="KT16")
            Q8 = [
                qt_pool.tile([P, 8, CH], f8, tag=f"qt{i}", name=f"Q8_{i}")
                for i in range(NSLOT)
            ]
            Q16 = qt_pool.tile([P, 8, CH], f16, tag="qt16", name="Q16")
            V16 = vt_pool.tile([P, 2, D], f16, tag="v16", name="V16")
            V8 = vt_pool.tile([P, 32, D], f8, tag="v8", name="V8")
            dmat = const_pool.tile([P, CH], f16, tag="dmat")
            amat = const_pool.tile([P, 32], f16, tag="amat")
            ones16 = const_pool.tile([P, P], f16, tag="ones16")
            ones8 = const_pool.tile([P, 2, P], f8, tag="ones8")
            bias8 = const_pool.tile([P, 1], f32, tag="bias8")
            nc.gpsimd.dma_start(out=dmat[:], in_=dmat_d[:])
            nc.gpsimd.dma_start(out=amat[:], in_=amat_d[:])
            nc.gpsimd.dma_start(out=bias8[:], in_=bias8_d[:])
            nc.gpsimd.dma_start(out=ones16[:], in_=ones16_d[:])
            nc.gpsimd.dma_start(out=ones8[:], in_=ones8_d[:])

            # ---------- Phase A: projections ------------------------------
            with (
                tc.tile_pool(name="w16", bufs=3) as w16_pool,
                tc.tile_pool(name="w8", bufs=1) as w8_pool,
                tc.tile_pool(name="x16", bufs=1) as x16_pool,
                tc.tile_pool(name="x8", bufs=2) as x8_pool,
                tc.tile_pool(name="xq8", bufs=2) as xq8_pool,
                tc.tile_pool(name="ps0", bufs=8, space="PSUM") as ps_pool,
            ):
                wk16 = w16_pool.tile([P, 8, D], f16, tag="w16", name="wk16")
                wk8 = w8_pool.tile([P, 8, D], f8, tag="wk8")
                wv8 = w8_pool.tile([P, 8, D], f8, tag="wv8")
                wq8 = w8_pool.tile([P, 8, D], f8, tag="wq8")
                xc16 = x16_pool.tile([P, 8, CH], f16, tag="xc16")
                xq16 = x16_pool.tile([P, 8, CH], f16, tag="xq16")

                # slab-split loads so the first matmuls start early.
                # Order: Q16 runs first (wq16+xq16 land first on their
                # queues), then K island (wk16), then V island (wv16 reuses
                # wq16's ring slot once Q16 is done).
                xc8 = xq8_pool.tile([P, 8, CH], f8, tag="xq", name="xc8")
                wq16 = w16_pool.tile([P, 8, D], f16, tag="w16", name="wq16")
                wv16 = w16_pool.tile([P, 8, D], f16, tag="w16", name="wv16")
                # fp8-first: small fp8 loads go ahead; bulky fp16 weights
                # stream during ~40us of fp8 chunk compute
                nc.sync.dma_start(out=wk8[:], in_=wk8_d[:])
                nc.gpsimd.dma_start(out=wv8[:], in_=wv8_d[:])
                xc_pre = []
                for i in range(2):
                    xc = x8_pool.tile([P, 8, CH], f8, tag="xc", name=f"xcp{i}")
                    (nc.sync if i == 0 else nc.scalar).dma_start(
                        out=xc[:], in_=x8_d[i]
                    )
                    xc_pre.append(xc)
                for d in range(8):
                    nc.sync.dma_start(out=wq16[:, d, :], in_=wq16_d[:, d, :])
                    nc.scalar.dma_start(out=xq16[:, d, :], in_=xq16_d[:, d, :])
                for d in range(8):
                    nc.sync.dma_start(out=wk16[:, d, :], in_=wk16_d[:, d, :])
                    nc.gpsimd.dma_start(out=xc16[:, d, :], in_=xc16_d[:, d, :])
                for d in range(8):
                    nc.sync.dma_start(out=wv16[:, d, :], in_=wv16_d[:, d, :])
                nc.scalar.dma_start(out=xc8[:], in_=xc8_d[:])
                nc.scalar.dma_start(out=wq8[:], in_=wq8_d[:])

                # fp8 chunks 1-3 first (x tiles 1-2 pre-loaded)
                for c in range(1, 4):
                    if c <= 2:
                        xc = xc_pre[c - 1]
                    else:
                        xc = x8_pool.tile([P, 8, CH], f8, tag="xc", name=f"xc{c}")
                        nc.scalar.dma_start(out=xc[:], in_=x8_d[c - 1])
                    for o in range(8):
                        ps = ps_pool.tile([P, CH], f32, tag="ps", name="psk8")
                        for d2 in range(4):
                            nc.tensor.matmul(
                                ps[:],
                                lhsT=wk8[:, ds(2 * d2, 2), ts(o, P)],
                                rhs=xc[:, ds(2 * d2, 2), :],
                                start=(d2 == 0),
                                stop=(d2 == 3),
                                perf_mode=DR,
                            )
                        nc.vector.tensor_copy(
                            KT8[c // 2][:, o, ds((c % 2) * CH, CH)], ps[:]
                        )
                    for kt in range(4):
                        for oh in range(2):
                            ps = ps_pool.tile([P, CH], f32, tag="ps", name="psv8")
                            for d2 in range(4):
                                nc.tensor.matmul(
                                    ps[:],
                                    lhsT=xc[:, ds(2 * d2, 2), ts(kt, P)],
                                    rhs=wv8[:, ds(2 * d2, 2), ts(oh, CH)],
                                    start=(d2 == 0),
                                    stop=(d2 == 3),
                                    perf_mode=DR,
                                )
                            nc.scalar.copy(
                                V8[:, 4 * c + kt, ts(oh, CH)], ps[:]
                            )

                # Q16 projection first (fp16 slot 0, dual store).
                # d-outer over 8 psum banks: each arriving wq16 slab feeds 8
                # matmuls, so the PE starts after slab 0 instead of pacing
                # behind the whole weight load.
                qps = [
                    ps_pool.tile([P, CH], f32, tag="ps", name=f"psq16_{o}")
                    for o in range(8)
                ]
                for d in range(8):
                    for o in range(8):
                        nc.tensor.matmul(
                            qps[o][:],
                            lhsT=wq16[:, d, ts(o, P)],
                            rhs=xq16[:, d, :],
                            start=(d == 0),
                            stop=(d == 7),
                        )
                for o in range(8):
                    nc.vector.tensor_copy(Q16[:, o, :], qps[o][:])
                    nc.scalar.activation(
                        Q8[0][:, o, :], qps[o][:], Copy, scale=32.0
                    )

                # fp16 chunk-0 K (dual store: f16 true + f8 x32)
                for o in range(8):
                    ps = ps_pool.tile([P, CH], f32, tag="ps", name="psk16")
                    for d in range(8):
                        nc.tensor.matmul(
                            ps[:],
                            lhsT=wk16[:, d, ts(o, P)],
                            rhs=xc16[:, d, :],
                            start=(d == 0),
                            stop=(d == 7),
                        )
                    nc.vector.tensor_copy(KT16[:, o, :], ps[:])
                    nc.scalar.activation(
                        KT8[0][:, o, ds(0, CH)], ps[:], Copy, scale=32.0
                    )
                # chunk-0 V: kt 0-1 fp16 (dual store f16 + f8 x16),
                # kt 2-3 fp8 DoubleRow (fp8 x, fp8 w)
                for kt in range(2):
                    for oh in range(2):
                        ps = ps_pool.tile([P, CH], f32, tag="ps", name="psv16")
                        for d in range(8):
                            nc.tensor.matmul(
                                ps[:],
                                lhsT=xc16[:, d, ts(kt, P)],
                                rhs=wv16[:, d, ts(oh, CH)],
                                start=(d == 0),
                                stop=(d == 7),
                            )
                        nc.vector.tensor_copy(V16[:, kt, ts(oh, CH)], ps[:])
                        nc.scalar.activation(
                            V8[:, kt, ts(oh, CH)], ps[:], Copy, scale=16.0
                        )
                for kt in range(2, 4):
                    for oh in range(2):
                        ps = ps_pool.tile([P, CH], f32, tag="ps", name="psv8c0")
                        for d2 in range(4):
                            nc.tensor.matmul(
                                ps[:],
                                lhsT=xc8[:, ds(2 * d2, 2), ts(kt, P)],
                                rhs=wv8[:, ds(2 * d2, 2), ts(oh, CH)],
                                start=(d2 == 0),
                                stop=(d2 == 3),
                                perf_mode=DR,
                            )
                        nc.scalar.copy(V8[:, kt, ts(oh, CH)], ps[:])

                # fp8 chunks 4-7
                for c in range(4, 8):
                    xc = x8_pool.tile([P, 8, CH], f8, tag="xc", name=f"xc{c}")
                    nc.sync.dma_start(out=xc[:], in_=x8_d[c - 1])
                    for o in range(8):
                        ps = ps_pool.tile([P, CH], f32, tag="ps", name="psk8")
                        for d2 in range(4):
                            nc.tensor.matmul(
                                ps[:],
                                lhsT=wk8[:, ds(2 * d2, 2), ts(o, P)],
                                rhs=xc[:, ds(2 * d2, 2), :],
                                start=(d2 == 0),
                                stop=(d2 == 3),
                                perf_mode=DR,
                            )
                        nc.vector.tensor_copy(
                            KT8[c // 2][:, o, ds((c % 2) * CH, CH)], ps[:]
                        )
                    for kt in range(4):
                        for oh in range(2):
                            ps = ps_pool.tile([P, CH], f32, tag="ps", name="psv8")
                            for d2 in range(4):
                                nc.tensor.matmul(
                                    ps[:],
                                    lhsT=xc[:, ds(2 * d2, 2), ts(kt, P)],
                                    rhs=wv8[:, ds(2 * d2, 2), ts(oh, CH)],
                                    start=(d2 == 0),
                                    stop=(d2 == 3),
                                    perf_mode=DR,
                                )
                            nc.scalar.copy(
                                V8[:, 4 * c + kt, ts(oh, CH)], ps[:]
                            )

                # Q projections: fp8 slots 1-3
                for sl in range(1, 4):
                    xq = xq8_pool.tile([P, 8, CH], f8, tag="xq", name=f"xq{sl}")
                    nc.scalar.dma_start(out=xq[:], in_=xq8_d[sl - 1])
                    for o in range(8):
                        ps = ps_pool.tile([P, CH], f32, tag="ps", name="psq8")
                        for d2 in range(4):
                            nc.tensor.matmul(
                                ps[:],
                                lhsT=wq8[:, ds(2 * d2, 2), ts(o, P)],
                                rhs=xq[:, ds(2 * d2, 2), :],
                                start=(d2 == 0),
                                stop=(d2 == 3),
                                perf_mode=DR,
                            )
                        nc.vector.tensor_copy(Q8[sl][:, o, :], ps[:])

            # ---------------- Phase B: attention --------------------------
            with (
                tc.tile_pool(name="p16", bufs=2) as p16_pool,
                tc.tile_pool(name="p8", bufs=30) as p8_pool,
                tc.tile_pool(name="et", bufs=6) as e_pool,
                tc.tile_pool(name="fo", bufs=6) as f_pool,
                tc.tile_pool(name="pss", bufs=4, space="PSUM") as s_ps_pool,
                tc.tile_pool(name="psc", bufs=3, space="PSUM") as c_ps_pool,
                tc.tile_pool(name="psd", bufs=1, space="PSUM") as d_ps_pool,
            ):
                for s in range(NSLOT):
                    nk = NK[s]
                    n16 = 2 if s == 0 else 0   # fp16 P/V tiles (slot 0 only)
                    np8 = (nk - n16) // 2
                    P16 = (
                        p16_pool.tile([P, 2, CH], f16, tag="p16", name="P16")
                        if n16
                        else None
                    )
                    P8 = [
                        p8_pool.tile([P, 2, CH], f8, tag="p8", name=f"P8_{s}_{t}")
                        for t in range(np8)
                    ]
                    jorder = (
                        list(range(nk))
                        if s == 0
                        else list(range(nk - 8, nk)) + list(range(nk - 8))
                    )
                    for j in jorder:
                        mm16 = (s == 0 and j < 4)
                        # Diagonal tiles (last 4 of each slot; tiles 4-7 in
                        # slot 0): queries below the tile start are masked,
                        # so skip those score columns and zero the P region.
                        di = -1
                        if s == 0 and j >= 4:
                            di = j - 4
                        elif s > 0 and j >= nk - 4:
                            di = j - (nk - 4)
                        qlo = P * di if di > 0 else 0
                        CHq = CH - qlo
                        sps = s_ps_pool.tile([P, CH], f32, name="sps")
                        if mm16:
                            for o in range(8):
                                nc.tensor.matmul(
                                    sps[:],
                                    lhsT=KT16[:, o, ds(j * P, P)],
                                    rhs=Q16[:, o, :],
                                    start=(o == 0),
                                    stop=(o == 7),
                                )
                        else:
                            for d2 in range(4):
                                nc.tensor.matmul(
                                    sps[:, ds(qlo, CHq)],
                                    lhsT=KT8[j // 8][
                                        :, ds(2 * d2, 2), ds((j % 8) * P, P)
                                    ],
                                    rhs=Q8[s][:, ds(2 * d2, 2), ds(qlo, CHq)],
                                    start=(d2 == 0),
                                    stop=(d2 == 3),
                                    perf_mode=DR,
                                )
                        scale = SC16 if mm16 else SC8
                        if j < n16:
                            dst = P16[:, j, :]
                            bias = 0.0
                        else:
                            dst = P8[(j - n16) // 2][:, (j - n16) % 2, :]
                            bias = bias8[:]
                        stt = (s == 0) or (j >= nk - 8)
                        if qlo:
                            nc.gpsimd.memset(dst[:, ds(0, qlo)], 0.0)
                            dst = dst[:, ds(qlo, CHq)]
                        if stt:
                            et = e_pool.tile([P, CH], f16, tag="et", name="et")
                            nc.scalar.activation(
                                et[:, ds(qlo, CHq)],
                                sps[:, ds(qlo, CHq)],
                                Exp,
                                scale=scale,
                                bias=bias,
                            )
                            col = j if s == 0 else 8 * s + (j - (nk - 8))
                            nc.vector.scalar_tensor_tensor(
                                out=dst,
                                in0=dmat[:, ds(qlo, CHq)],
                                scalar=amat[:, ds(col, 1)],
                                in1=et[:, ds(qlo, CHq)],
                                op0=is_le,
                                op1=mult,
                            )
                        else:
                            nc.scalar.activation(
                                dst, sps[:], Exp, scale=scale, bias=bias
                            )
                    # denominator, replicated across partitions by the
                    # full-width ones lhsT (no broadcast matmul needed)
                    dps = d_ps_pool.tile([P, CH], f32, name="dps")
                    for j in range(n16):
                        nc.tensor.matmul(
                            dps[:],
                            lhsT=ones16[:],
                            rhs=P16[:, j, :],
                            start=(j == 0),
                            stop=False,
                        )
                    for t in range(np8):
                        # last pair (tiles nk-2/nk-1): P is zero for q<256
                        ql = CH // 2 if t == np8 - 1 else 0
                        nc.tensor.matmul(
                            dps[:, ds(ql, CH - ql)],
                            lhsT=ones8[:],
                            rhs=P8[t][:, :, ds(ql, CH - ql)],
                            start=(n16 == 0 and t == 0),
                            stop=(t == np8 - 1),
                            perf_mode=DR,
                        )
                    rec = f_pool.tile([P, CH], f32, tag="rec", name="rec")
                    nc.vector.reciprocal(rec[:], dps[:])
                    # context
                    for o in range(8):
                        cps = c_ps_pool.tile([P, CH], f32, name="cps")
                        for j in range(n16):
                            nc.tensor.matmul(
                                cps[:],
                                lhsT=V16[:, j, ts(o, P)],
                                rhs=P16[:, j, :],
                                start=(j == 0),
                                stop=False,
                            )
                        for t in range(np8):
                            ql = CH // 2 if t == np8 - 1 else 0
                            nc.tensor.matmul(
                                cps[:, ds(ql, CH - ql)],
                                lhsT=V8[:, ds(n16 + 2 * t, 2), ts(o, P)],
                                rhs=P8[t][:, :, ds(ql, CH - ql)],
                                start=(n16 == 0 and t == 0),
                                stop=(t == np8 - 1),
                                perf_mode=DR,
                            )
                        ft = f_pool.tile([P, CH], f32, tag="ft", name="ft")
                        nc.vector.tensor_mul(ft[:], cps[:], rec[:])
                        qeng = (nc.sync, nc.scalar, nc.gpsimd)[o % 3]
                        qeng.dma_start(
                            out=outT[ds(o * P, P), ts(s, CH)], in_=ft[:]
                        )

    nc.compile()
    return nc


def _get_program():
    global _PROGRAM
    if _PROGRAM is None:
        _PROGRAM = _build_program()
    return _PROGRAM


def _tile_w(w, scale, dtype):
    # [o, i] -> [p, d_slab, o] with d = d_slab*128 + p
    wt = (np.asarray(w, dtype=np.float32).T * scale).astype(dtype)
    return np.ascontiguousarray(wt.reshape(8, P, D).transpose(1, 0, 2))


def _tile_x(xt, dtype):
    # [d, s_cols] -> [p, d_slab, s] (or [n, p, d_slab, s] for multi-chunk)
    ncols = xt.shape[1]
    nch = ncols // CH
    t = np.ascontiguousarray(
        xt.reshape(8, P, nch, CH).transpose(2, 1, 0, 3)
    ).astype(dtype)
    if nch == 1:
        return np.ascontiguousarray(t[0])
    return t


def _make_in_maps(x, W_query, W_key, W_value):
    xT = np.asarray(x, dtype=np.float32).transpose(0, 2, 1)  # [B, D, S]

    wk16 = _tile_w(W_key, 1.0, np.float16)
    wv16 = _tile_w(W_value, 1.0, np.float16)
    wq16 = _tile_w(W_query, 1.0, np.float16)
    wk8 = _tile_w(W_key, 32.0, F8)
    wv8 = _tile_w(W_value, 16.0, F8)
    wq8 = _tile_w(W_query, 32.0, F8)

    dmat = (
        np.arange(P, dtype=np.float32)[:, None]
        - np.arange(CH, dtype=np.float32)[None, :]
    ).astype(np.float16)
    amat_h = []
    for h in range(2):
        a = np.full((P, 32), -32768.0, np.float32)
        for sl in range(NSLOT):
            cid = CHUNKS_H[h][sl]
            nk = NK[sl]
            if sl == 0:
                for j in range(8):
                    a[:, j] = CH * cid - P * j
            else:
                for j in range(nk - 8, nk):
                    a[:, 8 * sl + (j - (nk - 8))] = CH * cid - P * j
        amat_h.append(np.ascontiguousarray(a.astype(np.float16)))
    ones16 = np.ones((P, P), np.float16)
    ones8 = np.full((P, 2, P), 16.0, F8)
    bias8 = np.full((P, 1), BIAS8, np.float32)

    in_maps = []
    for core in range(8):
        b, h = core // 2, core % 2
        xb = xT[b]
        q0 = CHUNKS_H[h][0]
        xq_cols = np.concatenate(
            [np.arange(c * CH, (c + 1) * CH) for c in CHUNKS_H[h][1:]]
        )
        in_maps.append(
            {
                "xc16": _tile_x(xb[:, :CH], np.float16),
                "xc8": _tile_x(xb[:, :CH], F8),
                "x8": _tile_x(xb[:, CH:], F8),
                "xq16": _tile_x(xb[:, q0 * CH : (q0 + 1) * CH], np.float16),
                "xq8": _tile_x(np.ascontiguousarray(xb[:, xq_cols]), F8),
                "wk16": wk16,
                "wv16": wv16,
                "wq16": wq16,
                "wk8": wk8,
                "wv8": wv8,
                "wq8": wq8,
                "amat": amat_h[h],
                "dmat": dmat,
                "ones16": ones16,
                "ones8": ones8,
                "bias8": bias8,
            }
        )
    return in_maps


def _assemble(results):
    out = np.empty((B, S, D), np.float32)
    for core in range(8):
        b, h = core // 2, core % 2
        oT = np.asarray(results[core]["outT"])  # [D, NQ]
        for slot, c in enumerate(CHUNKS_H[h]):
            out[b, c * CH : (c + 1) * CH, :] = oT[:, slot * CH : (slot + 1) * CH].T
    return out


def run(inputs, trace=False, trace_cores=None):
    """Run the kernel; returns (output, BassKernelResults)."""
    from concourse.bass_utils import run_bass_kernel_spmd

    nc = _get_program()
    in_maps = _make_in_maps(
        inputs["x"], inputs["W_query"], inputs["W_key"], inputs["W_value"]
    )
    kw = {}
    if trace:
        kw = dict(trace=True, trace_cores=trace_cores, stitch_traces=False)
    res = run_bass_kernel_spmd(nc, in_maps, list(range(8)), **kw)
    return _assemble(res.results), res


def kernel(x, W_query, W_key, W_value):
    out, _ = run({"x": x, "W_query": W_query, "W_key": W_key, "W_value": W_value})
    return out


# revision 35
# speedup vs baseline: 1.1213x; 1.0007x over previous
"""Causal single-head attention (B=4, S=4096, D=1024) on 8 TRN2 NeuronCores.

Sharding: core = (batch b, half h).  Each core computes attention output for
2048 queries of one batch: query chunks {0,3,4,7} (h=0) or {1,2,5,6} (h=1) of
8x512, which balances causal work.  Each core projects K^T/V for its full
batch; everything stays SBUF-resident (no DRAM scratch).

Mixed precision (validated in numpy + CoreSim, rel ~2.5e-3 vs 2e-2 gate):
  - fp16 island: scores for (q<512, k<512); ctx P/V fp16 only for slot 0
    (queries<1024) x keys<512.  Protects early (few-key) rows where softmax
    averaging is weak; all other queries are averaging-protected.
  - everything else: fp8 e4m3 with DoubleRow matmuls (2x PE throughput).
Scale folding (dodges e4m3 subnormals/overflow):
  Wq8,Wk8 scaled x32 -> s8 = 1024*s -> exp scale 1/32768
  Wv8 scaled x16; P8 stored as p/16 via exp bias -ln(16); den repaired with
  ones8=16; fp16 paths are true-scale.
Denominator: ones-column matmuls with full-partition lhsT produce den
replicated over all 128 partitions (no separate broadcast matmul needed).
Layouts (all SBUF):
  K^T  : KT8 4x[P,8,1024] f8, KT16 [P,8,512] f16
  Q^T  : Q8 4x[P,8,512] f8 per slot, Q16 [P,8,512] f16 (slot 0)
  V    : V8 [P,32,1024] f8 (all tiles), V16 [P,4,1024] f16 (tiles 0-3)
  P    : P16 [P,4,512] f16 (slot 0 tiles 0-3), P8 pairs [P,2,512] f8
  scores^T = [k, q]: psum = sum_d KT[d,k128].T @ QT[d,q512] (no transposes)
"""

import math
import sys

for _p in ("/opt/trn_rl_repo",):
    if _p not in sys.path:
        sys.path.insert(0, _p)

import numpy as np
import ml_dtypes

B, S, D = 4, 4096, 1024
P = 128
CH = 512                       # query chunk
NSLOT = 4                      # chunks per core
NQ = NSLOT * CH                # queries per core
NK = [8, 16, 24, 32]           # k-tiles per slot (uniform across cores)
CHUNKS_H = [[0, 3, 4, 7], [1, 2, 5, 6]]
SC16 = 1.0 / 32.0              # 1/sqrt(D)
SC8 = 1.0 / 32768.0            # 1/sqrt(D) / (32*32)
BIAS8 = -math.log(16.0)        # P8 stored as p/16
F8 = ml_dtypes.float8_e4m3

_PROGRAM = None


def _build_program():
    import concourse.bass as bass
    import concourse.tile as tile
    import concourse.mybir as mybir
    from concourse import bacc
    from concourse.bass import ds, ts

    f32 = mybir.dt.float32
    f16 = mybir.dt.float16
    f8 = mybir.dt.float8e4
    DR = mybir.MatmulPerfMode.DoubleRow

    nc = bacc.Bacc(trn_type="TRN2", target_bir_lowering=False, debug=False,
                   num_devices=8)

    xc16_d = nc.declare_dram_parameter("xc16", [P, 8, CH], f16, isOutput=False)
    x8_d = nc.declare_dram_parameter("x8", [7, P, 8, CH], f8, isOutput=False)
    xc8_d = nc.declare_dram_parameter("xc8", [P, 8, CH], f8, isOutput=False)
    xq16_d = nc.declare_dram_parameter("xq16", [P, 8, CH], f16, isOutput=False)
    xq8_d = nc.declare_dram_parameter("xq8", [3, P, 8, CH], f8, isOutput=False)
    wk16_d = nc.declare_dram_parameter("wk16", [P, 8, D], f16, isOutput=False)
    wv16_d = nc.declare_dram_parameter("wv16", [P, 8, D], f16, isOutput=False)
    wq16_d = nc.declare_dram_parameter("wq16", [P, 8, D], f16, isOutput=False)
    wk8_d = nc.declare_dram_parameter("wk8", [P, 8, D], f8, isOutput=False)
    wv8_d = nc.declare_dram_parameter("wv8", [P, 8, D], f8, isOutput=False)
    wq8_d = nc.declare_dram_parameter("wq8", [P, 8, D], f8, isOutput=False)
    amat_d = nc.declare_dram_parameter("amat", [P, 32], f16, isOutput=False)
    bias8_d = nc.declare_dram_parameter("bias8", [P, 1], f32, isOutput=False)
    dmat_d = nc.declare_dram_parameter("dmat", [P, CH], f16, isOutput=False)
    ones16_d = nc.declare_dram_parameter("ones16", [P, P], f16, isOutput=False)
    ones8_d = nc.declare_dram_parameter("ones8", [P, 2, P], f8, isOutput=False)
    outT = nc.declare_dram_parameter("outT", [D, NQ], f32, isOutput=True)

    Exp = mybir.ActivationFunctionType.Exp
    Copy = mybir.ActivationFunctionType.Copy
    is_le = mybir.AluOpType.is_le
    mult = mybir.AluOpType.mult

    with tile.TileContext(nc, pool_alloc_mode="queue") as tc:
        with (
            tc.tile_pool(name="kt", bufs=1) as kt_pool,
            tc.tile_pool(name="qt", bufs=1) as qt_pool,
            tc.tile_pool(name="vt", bufs=1) as vt_pool,
            tc.tile_pool(name="const", bufs=1) as const_pool,
        ):
            KT8 = [
                kt_pool.tile([P, 8, 1024], f8, tag=f"kt{i}", name=f"KT8_{i}")
                for i in range(4)
            ]
            KT16 = kt_pool.tile([P, 8, CH], f16, tag="kt16", name="KT16")
            Q8 = [
                qt_pool.tile([P, 8, CH], f8, tag=f"qt{i}", name=f"Q8_{i}")
                for i in range(NSLOT)
            ]
            Q16 = qt_pool.tile([P, 8, CH], f16, tag="qt16", name="Q16")
            V16 = vt_pool.tile([P, 2, D], f16, tag="v16", name="V16")
            V8 = vt_pool.tile([P, 32, D], f8, tag="v8", name="V8")
            dmat = const_pool.tile([P, CH], f16, tag="dmat")
            amat = const_pool.tile([P, 32], f16, tag="amat")
            ones16 = const_pool.tile([P, P], f16, tag="ones16")
            ones8 = const_pool.tile([P, 2, P], f8, tag="ones8")
            bias8 = const_pool.tile([P, 1], f32, tag="bias8")
            nc.gpsimd.dma_start(out=dmat[:], in_=dmat_d[:])
            nc.gpsimd.dma_start(out=amat[:], in_=amat_d[:])
            nc.gpsimd.dma_start(out=bias8[:], in_=bias8_d[:])
            nc.gpsimd.dma_start(out=ones16[:], in_=ones16_d[:])
            nc.gpsimd.dma_start(out=ones8[:], in_=ones8_d[:])

            # ---------- Phase A: projections ------------------------------
            with (
                tc.tile_pool(name="w16", bufs=2) as w16_pool,
                tc.tile_pool(name="w8", bufs=1) as w8_pool,
                tc.tile_pool(name="x16", bufs=1) as x16_pool,
                tc.tile_pool(name="x8", bufs=3) as x8_pool,
                tc.tile_pool(name="xq8", bufs=2) as xq8_pool,
                tc.tile_pool(name="ps0", bufs=8, space="PSUM") as ps_pool,
            ):
                wk16 = w16_pool.tile([P, 8, D], f16, tag="w16", name="wk16")
                wk8 = w8_pool.tile([P, 8, D], f8, tag="wk8")
                wv8 = w8_pool.tile([P, 8, D], f8, tag="wv8")
                wq8 = w8_pool.tile([P, 8, D], f8, tag="wq8")
                xc16 = x16_pool.tile([P, 8, CH], f16, tag="xc16")
                xq16 = x16_pool.tile([P, 8, CH], f16, tag="xq16")

                # slab-split loads so the first matmuls start early.
                # Order: Q16 runs first (wq16+xq16 land first on their
                # queues), then K island (wk16), then V island (wv16 reuses
                # wq16's ring slot once Q16 is done).
                xc8 = xq8_pool.tile([P, 8, CH], f8, tag="xq", name="xc8")
                wq16 = w16_pool.tile([P, 8, D], f16, tag="w16", name="wq16")
                wv16 = w16_pool.tile([P, 8, D], f16, tag="w16", name="wv16")
                # fp8-first: small fp8 loads go ahead; bulky fp16 weights
                # stream during ~40us of fp8 chunk compute
                nc.sync.dma_start(out=wk8[:], in_=wk8_d[:])
                nc.gpsimd.dma_start(out=wv8[:], in_=wv8_d[:])
                xc_pre = []
                for i in range(3):
                    xc = x8_pool.tile([P, 8, CH], f8, tag="xc", name=f"xcp{i}")
                    (nc.sync, nc.scalar, nc.gpsimd)[i].dma_start(
                        out=xc[:], in_=x8_d[i]
                    )
                    xc_pre.append(xc)
                for d in range(8):
                    nc.sync.dma_start(out=wq16[:, d, :], in_=wq16_d[:, d, :])
                    nc.scalar.dma_start(out=xq16[:, d, :], in_=xq16_d[:, d, :])
                for d in range(8):
                    nc.sync.dma_start(out=wk16[:, d, :], in_=wk16_d[:, d, :])
                    nc.gpsimd.dma_start(out=xc16[:, d, :], in_=xc16_d[:, d, :])
                for d in range(8):
                    nc.sync.dma_start(out=wv16[:, d, :], in_=wv16_d[:, d, :])
                nc.scalar.dma_start(out=xc8[:], in_=xc8_d[:])
                nc.scalar.dma_start(out=wq8[:], in_=wq8_d[:])

                # fp8 chunks 1-3 first (x tiles 1-2 pre-loaded)
                for c in range(1, 4):
                    xc = xc_pre[c - 1]
                    for o in range(8):
                        ps = ps_pool.tile([P, CH], f32, tag="ps", name="psk8")
                        for d2 in range(4):
                            nc.tensor.matmul(
                                ps[:],
                                lhsT=wk8[:, ds(2 * d2, 2), ts(o, P)],
                                rhs=xc[:, ds(2 * d2, 2), :],
                                start=(d2 == 0),
                                stop=(d2 == 3),
                                perf_mode=DR,
                            )
                        nc.vector.tensor_copy(
                            KT8[c // 2][:, o, ds((c % 2) * CH, CH)], ps[:]
                        )
                    for kt in range(4):
                        for oh in range(2):
                            ps = ps_pool.tile([P, CH], f32, tag="ps", name="psv8")
                            for d2 in range(4):
                                nc.tensor.matmul(
                                    ps[:],
                                    lhsT=xc[:, ds(2 * d2, 2), ts(kt, P)],
                                    rhs=wv8[:, ds(2 * d2, 2), ts(oh, CH)],
                                    start=(d2 == 0),
                                    stop=(d2 == 3),
                                    perf_mode=DR,
                                )
                            nc.scalar.copy(
                                V8[:, 4 * c + kt, ts(oh, CH)], ps[:]
                            )

                # Q16 projection first (fp16 slot 0, dual store).
                # d-outer over 8 psum banks: each arriving wq16 slab feeds 8
                # matmuls, so the PE starts after slab 0 instead of pacing
                # behind the whole weight load.
                qps = [
                    ps_pool.tile([P, CH], f32, tag="ps", name=f"psq16_{o}")
                    for o in range(8)
                ]
                for d in range(8):
                    for o in range(8):
                        nc.tensor.matmul(
                            qps[o][:],
                            lhsT=wq16[:, d, ts(o, P)],
                            rhs=xq16[:, d, :],
                            start=(d == 0),
                            stop=(d == 7),
                        )
                for o in range(8):
                    nc.vector.tensor_copy(Q16[:, o, :], qps[o][:])
                    nc.scalar.activation(
                        Q8[0][:, o, :], qps[o][:], Copy, scale=32.0
                    )

                # fp16 chunk-0 K (dual store: f16 true + f8 x32)
                for o in range(8):
                    ps = ps_pool.tile([P, CH], f32, tag="ps", name="psk16")
                    for d in range(8):
                        nc.tensor.matmul(
                            ps[:],
                            lhsT=wk16[:, d, ts(o, P)],
                            rhs=xc16[:, d, :],
                            start=(d == 0),
                            stop=(d == 7),
                        )
                    nc.vector.tensor_copy(KT16[:, o, :], ps[:])
                    nc.scalar.activation(
                        KT8[0][:, o, ds(0, CH)], ps[:], Copy, scale=32.0
                    )
                # chunk-0 V: kt 0-1 fp16 (dual store f16 + f8 x16),
                # kt 2-3 fp8 DoubleRow (fp8 x, fp8 w)
                for kt in range(2):
                    for oh in range(2):
                        ps = ps_pool.tile([P, CH], f32, tag="ps", name="psv16")
                        for d in range(8):
                            nc.tensor.matmul(
                                ps[:],
                                lhsT=xc16[:, d, ts(kt, P)],
                                rhs=wv16[:, d, ts(oh, CH)],
                                start=(d == 0),
                                stop=(d == 7),
                            )
                        nc.vector.tensor_copy(V16[:, kt, ts(oh, CH)], ps[:])
                        nc.scalar.activation(
                            V8[:, kt, ts(oh, CH)], ps[:], Copy, scale=16.0
                        )
                for kt in range(2, 4):
                    for oh in range(2):
                        ps = ps_pool.tile([P, CH], f32, tag="ps", name="psv8c0")
                        for d2 in range(4):
                            nc.tensor.matmul(
                                ps[:],
                                lhsT=xc8[:, ds(2 * d2, 2), ts(kt, P)],
                                rhs=wv8[:, ds(2 * d2, 2), ts(oh, CH)],
                                start=(d2 == 0),
                                stop=(d2 == 3),
                                perf_mode=DR,
                            )
                        nc.scalar.copy(V8[:, kt, ts(oh, CH)], ps[:])

                # fp8 chunks 4-7
                for c in range(4, 8):
                    xc = x8_pool.tile([P, 8, CH], f8, tag="xc", name=f"xc{c}")
                    nc.sync.dma_start(out=xc[:], in_=x8_d[c - 1])
                    for o in range(8):
                        ps = ps_pool.tile([P, CH], f32, tag="ps", name="psk8")
                        for d2 in range(4):
                            nc.tensor.matmul(
                                ps[:],
                                lhsT=wk8[:, ds(2 * d2, 2), ts(o, P)],
                                rhs=xc[:, ds(2 * d2, 2), :],
                                start=(d2 == 0),
                                stop=(d2 == 3),
                                perf_mode=DR,
                            )
                        nc.vector.tensor_copy(
                            KT8[c // 2][:, o, ds((c % 2) * CH, CH)], ps[:]
                        )
                    for kt in range(4):
                        for oh in range(2):
                            ps = ps_pool.tile([P, CH], f32, tag="ps", name="psv8")
                            for d2 in range(4):
                                nc.tensor.matmul(
                                    ps[:],
                                    lhsT=xc[:, ds(2 * d2, 2), ts(kt, P)],
                                    rhs=wv8[:, ds(2 * d2, 2), ts(oh, CH)],
                                    start=(d2 == 0),
                                    stop=(d2 == 3),
                                    perf_mode=DR,
                                )
                            nc.scalar.copy(
                                V8[:, 4 * c + kt, ts(oh, CH)], ps[:]
                            )

                # Q projections: fp8 slots 1-3
                for sl in range(1, 4):
                    xq = xq8_pool.tile([P, 8, CH], f8, tag="xq", name=f"xq{sl}")
                    nc.scalar.dma_start(out=xq[:], in_=xq8_d[sl - 1])
                    for o in range(8):
                        ps = ps_pool.tile([P, CH], f32, tag="ps", name="psq8")
                        for d2 in range(4):
                            nc.tensor.matmul(
                                ps[:],
                                lhsT=wq8[:, ds(2 * d2, 2), ts(o, P)],
                                rhs=xq[:, ds(2 * d2, 2), :],
                                start=(d2 == 0),
                                stop=(d2 == 3),
                                perf_mode=DR,
                            )
                        nc.vector.tensor_copy(Q8[sl][:, o, :], ps[:])

            # ---------------- Phase B: attention --------------------------
            with (
                tc.tile_pool(name="p16", bufs=2) as p16_pool,
                tc.tile_pool(name="p8", bufs=30) as p8_pool,
                tc.tile_pool(name="et", bufs=6) as e_pool,
                tc.tile_pool(name="fo", bufs=6) as f_pool,
                tc.tile_pool(name="pss", bufs=4, space="PSUM") as s_ps_pool,
                tc.tile_pool(name="psc", bufs=3, space="PSUM") as c_ps_pool,
                tc.tile_pool(name="psd", bufs=1, space="PSUM") as d_ps_pool,
            ):
                for s in range(NSLOT):
                    nk = NK[s]
                    n16 = 2 if s == 0 else 0   # fp16 P/V tiles (slot 0 only)
                    np8 = (nk - n16) // 2
                    P16 = (
                        p16_pool.tile([P, 2, CH], f16, tag="p16", name="P16")
                        if n16
                        else None
                    )
                    P8 = [
                        p8_pool.tile([P, 2, CH], f8, tag="p8", name=f"P8_{s}_{t}")
                        for t in range(np8)
                    ]
                    jorder = (
                        list(range(nk))
                        if s == 0
                        else list(range(nk - 8, nk)) + list(range(nk - 8))
                    )
                    for j in jorder:
                        mm16 = (s == 0 and j < 4)
                        # Diagonal tiles (last 4 of each slot; tiles 4-7 in
                        # slot 0): queries below the tile start are masked,
                        # so skip those score columns and zero the P region.
                        di = -1
                        if s == 0 and j >= 4:
                            di = j - 4
                        elif s > 0 and j >= nk - 4:
                            di = j - (nk - 4)
                        qlo = P * di if di > 0 else 0
                        CHq = CH - qlo
                        sps = s_ps_pool.tile([P, CH], f32, name="sps")
                        if mm16:
                            for o in range(8):
                                nc.tensor.matmul(
                                    sps[:],
                                    lhsT=KT16[:, o, ds(j * P, P)],
                                    rhs=Q16[:, o, :],
                                    start=(o == 0),
                                    stop=(o == 7),
                                )
                        else:
                            for d2 in range(4):
                                nc.tensor.matmul(
                                    sps[:, ds(qlo, CHq)],
                                    lhsT=KT8[j // 8][
                                        :, ds(2 * d2, 2), ds((j % 8) * P, P)
                                    ],
                                    rhs=Q8[s][:, ds(2 * d2, 2), ds(qlo, CHq)],
                                    start=(d2 == 0),
                                    stop=(d2 == 3),
                                    perf_mode=DR,
                                )
                        scale = SC16 if mm16 else SC8
                        if j < n16:
                            dst = P16[:, j, :]
                            bias = 0.0
                        else:
                            dst = P8[(j - n16) // 2][:, (j - n16) % 2, :]
                            bias = bias8[:]
                        stt = (s == 0) or (j >= nk - 8)
                        if qlo:
                            nc.gpsimd.memset(dst[:, ds(0, qlo)], 0.0)
                            dst = dst[:, ds(qlo, CHq)]
                        if stt:
                            et = e_pool.tile([P, CH], f16, tag="et", name="et")
                            nc.scalar.activation(
                                et[:, ds(qlo, CHq)],
                                sps[:, ds(qlo, CHq)],
                                Exp,
                                scale=scale,
                                bias=bias,
                            )
                            col = j if s == 0 else 8 * s + (j - (nk - 8))
                            nc.vector.scalar_tensor_tensor(
                                out=dst,
                                in0=dmat[:, ds(qlo, CHq)],
                                scalar=amat[:, ds(col, 1)],
                                in1=et[:, ds(qlo, CHq)],
                                op0=is_le,
                                op1=mult,
                            )
                        else:
                            nc.scalar.activation(
                                dst, sps[:], Exp, scale=scale, bias=bias
                            )
                    # denominator, replicated across partitions by the
                    # full-width ones lhsT (no broadcast matmul needed)
                    dps = d_ps_pool.tile([P, CH], f32, name="dps")
                    for j in range(n16):
                        nc.tensor.matmul(
                            dps[:],
                            lhsT=ones16[:],
                            rhs=P16[:, j, :],
                            start=(j == 0),
                            stop=False,
                        )
                    for t in range(np8):
                        # last pair (tiles nk-2/nk-1): P is zero for q<256
                        ql = CH // 2 if t == np8 - 1 else 0
                        nc.tensor.matmul(
                            dps[:, ds(ql, CH - ql)],
                            lhsT=ones8[:],
                            rhs=P8[t][:, :, ds(ql, CH - ql)],
                            start=(n16 == 0 and t == 0),
                            stop=(t == np8 - 1),
                            perf_mode=DR,
                        )
                    rec = f_pool.tile([P, CH], f32, tag="rec", name="rec")
                    nc.vector.reciprocal(rec[:], dps[:])
                    # context
                    for o in range(8):
                        cps = c_ps_pool.tile([P, CH], f32, name="cps")
                        for j in range(n16):
                            nc.tensor.matmul(
                                cps[:],
                                lhsT=V16[:, j, ts(o, P)],
                                rhs=P16[:, j, :],
                                start=(j == 0),
                                stop=False,
                            )
                        for t in range(np8):
                            ql = CH // 2 if t == np8 - 1 else 0
                            nc.tensor.matmul(
                                cps[:, ds(ql, CH - ql)],
                                lhsT=V8[:, ds(n16 + 2 * t, 2), ts(o, P)],
                                rhs=P8[t][:, :, ds(ql, CH - ql)],
                                start=(n16 == 0 and t == 0),
                                stop=(t == np8 - 1),
                                perf_mode=DR,
                            )
                        ft = f_pool.tile([P, CH], f32, tag="ft", name="ft")
                        nc.vector.tensor_mul(ft[:], cps[:], rec[:])
                        qeng = (nc.sync, nc.scalar, nc.gpsimd)[o % 3]
                        qeng.dma_start(
                            out=outT[ds(o * P, P), ts(s, CH)], in_=ft[:]
                        )

    nc.compile()
    return nc


def _get_program():
    global _PROGRAM
    if _PROGRAM is None:
        _PROGRAM = _build_program()
    return _PROGRAM


def _tile_w(w, scale, dtype):
    # [o, i] -> [p, d_slab, o] with d = d_slab*128 + p
    wt = (np.asarray(w, dtype=np.float32).T * scale).astype(dtype)
    return np.ascontiguousarray(wt.reshape(8, P, D).transpose(1, 0, 2))


def _tile_x(xt, dtype):
    # [d, s_cols] -> [p, d_slab, s] (or [n, p, d_slab, s] for multi-chunk)
    ncols = xt.shape[1]
    nch = ncols // CH
    t = np.ascontiguousarray(
        xt.reshape(8, P, nch, CH).transpose(2, 1, 0, 3)
    ).astype(dtype)
    if nch == 1:
        return np.ascontiguousarray(t[0])
    return t


def _make_in_maps(x, W_query, W_key, W_value):
    xT = np.asarray(x, dtype=np.float32).transpose(0, 2, 1)  # [B, D, S]

    wk16 = _tile_w(W_key, 1.0, np.float16)
    wv16 = _tile_w(W_value, 1.0, np.float16)
    wq16 = _tile_w(W_query, 1.0, np.float16)
    wk8 = _tile_w(W_key, 32.0, F8)
    wv8 = _tile_w(W_value, 16.0, F8)
    wq8 = _tile_w(W_query, 32.0, F8)

    dmat = (
        np.arange(P, dtype=np.float32)[:, None]
        - np.arange(CH, dtype=np.float32)[None, :]
    ).astype(np.float16)
    amat_h = []
    for h in range(2):
        a = np.full((P, 32), -32768.0, np.float32)
        for sl in range(NSLOT):
            cid = CHUNKS_H[h][sl]
            nk = NK[sl]
            if sl == 0:
                for j in range(8):
                    a[:, j] = CH * cid - P * j
            else:
                for j in range(nk - 8, nk):
                    a[:, 8 * sl + (j - (nk - 8))] = CH * cid - P * j
        amat_h.append(np.ascontiguousarray(a.astype(np.float16)))
    ones16 = np.ones((P, P), np.float16)
    ones8 = np.full((P, 2, P), 16.0, F8)
    bias8 = np.full((P, 1), BIAS8, np.float32)

    in_maps = []
    for core in range(8):
        b, h = core // 2, core % 2
        xb = xT[b]
        q0 = CHUNKS_H[h][0]
        xq_cols = np.concatenate(
            [np.arange(c * CH, (c + 1) * CH) for c in CHUNKS_H[h][1:]]
        )
        in_maps.append(
            {
                "xc16": _tile_x(xb[:, :CH], np.float16),
                "xc8": _tile_x(xb[:, :CH], F8),
                "x8": _tile_x(xb[:, CH:], F8),
                "xq16": _tile_x(xb[:, q0 * CH : (q0 + 1) * CH], np.float16),
                "xq8": _tile_x(np.ascontiguousarray(xb[:, xq_cols]), F8),
                "wk16": wk16,
                "wv16": wv16,
                "wq16": wq16,
                "wk8": wk8,
                "wv8": wv8,
                "wq8": wq8,
                "amat": amat_h[h],
                "dmat": dmat,
                "ones16": ones16,
                "ones8": ones8,
                "bias8": bias8,
            }
        )
    return in_maps


def _assemble(results):
    out = np.empty((B, S, D), np.float32)
    for core in range(8):
        b, h = core // 2, core % 2
        oT = np.asarray(results[core]["outT"])  # [D, NQ]
        for slot, c in enumerate(CHUNKS_H[h]):
            out[b, c * CH : (c + 1) * CH, :] = oT[:, slot * CH : (slot + 1) * CH].T
    return out


def run(inputs, trace=False, trace_cores=None):
    """Run the kernel; returns (output, BassKernelResults)."""
    from concourse.bass_utils import run_bass_kernel_spmd

    nc = _get_program()
    in_maps = _make_in_maps(
        inputs["x"], inputs["W_query"], inputs["W_key"], inputs["W_value"]
    )
    kw = {}
    if trace:
        kw = dict(trace=True, trace_cores=trace_cores, stitch_traces=False)
    res = run_bass_kernel_spmd(nc, in_maps, list(range(8)), **kw)
    return _assemble(res.results), res


def kernel(x, W_query, W_key, W_value):
    out, _ = run({"x": x, "W_query": W_query, "W_key": W_key, "W_value": W_value})
    return out
